# revision 1
# baseline (speedup 1.0000x reference)
"""Trainium2 Bass kernel for nn_ClassifierAttn (single-query attention pooling).

Math restructuring (exact up to float assoc):
  Per (b,q) with e = video_enc[b,q] [S=2048, 768]:
    scores[s] = (e[s] . cv)/32 + const, cv = Wvp^T (Wk^T (Wq q)),
  const cancels in softmax, and
    pooled = ((softmax(scores) @ e) @ Wvp^T + bvp) @ Wv^T.
  So the big tensor is touched by exactly two mat-vec passes:
    (1) scores = e . cv   -- fused mul+reduce (scalar_tensor_tensor) on VectorE
    (2) ebar_un = p^T e   -- PE matmuls (p = exp(scores), unnormalized), with a
        ones-column appended to e so Z = sum(p) rides along in the same psum row
  Everything else is tiny [10, *] linear algebra (bf16 weights, f32 psum).

Engine plan per 128-row tile t of e:
  DVE: fused dot prod=(e*cv/32) (bf16 out) + row-sum -> scores column
  ACT: exp -> p column (bf16)
  PE:  two bf16 matmuls p^T @ prod accumulating [1,512]+[1,258] psum rows;
       prod's constant ones-columns make Z=sum(p) ride along in the last col.
The cv scaling inside prod is undone at evacuation by multiplying with 32/cv
(exact relative cancellation), so the bf16 "cast" of e is free - it IS the
dot-product's side output. All engines + DMA pipeline; HBM streaming paces.

Sharding: (B*QPV)=80 pairs, 10 per core across 8 cores; weights replicated
(shipped bf16), video stays f32 on the wire.
"""

import numpy as np

P = 128
NBQ = 10          # (b,q) pairs per core
SO = 16           # S / P
S = 2048
D = 768
H = 512
H2 = 1024
OUT = 5
N_CORES = 8
INV_SQRT = 1.0 / 32.0   # 1/sqrt(2H)
DW = 772          # 768 data + 2 ones cols + pad

_COMPILED = None
LAST_RESULTS = None


def ts(i, size):
    return slice(i * size, (i + 1) * size)


def _build():
    import concourse.bass as bass  # noqa: F401
    import concourse.mybir as mybir
    import concourse.tile as tile
    from concourse import bacc
    from concourse.masks import make_identity

    fp32 = mybir.dt.float32
    bf16 = mybir.dt.bfloat16
    Alu = mybir.AluOpType
    Act = mybir.ActivationFunctionType
    Ax = mybir.AxisListType

    nc = bacc.Bacc("TRN2", target_bir_lowering=False, debug=False,
                   num_devices=N_CORES)

    # ---- DRAM I/O (host-pre-tiled to DMA-contiguous layouts) ----
    ve = nc.dram_tensor("ve", [NBQ, P, SO, D], fp32, kind="ExternalInput")
    q0T = nc.dram_tensor("q0T", [P, 6, NBQ], bf16, kind="ExternalInput")
    omask = nc.dram_tensor("omask", [NBQ, OUT], fp32, kind="ExternalInput")
    WqpT = nc.dram_tensor("WqpT", [P, 6, H], bf16, kind="ExternalInput")
    WqT = nc.dram_tensor("WqT", [P, 4, H2], bf16, kind="ExternalInput")
    Wk = nc.dram_tensor("Wk", [P, 8, H], bf16, kind="ExternalInput")
    Wvp = nc.dram_tensor("Wvp", [P, 4, D], bf16, kind="ExternalInput")
    WvpT = nc.dram_tensor("WvpT", [P, 6, H], bf16, kind="ExternalInput")
    WvT = nc.dram_tensor("WvT", [P, 4, H], bf16, kind="ExternalInput")
    W1T = nc.dram_tensor("W1T", [P, 8, H], bf16, kind="ExternalInput")
    W2T = nc.dram_tensor("W2T", [P, 4, OUT], bf16, kind="ExternalInput")
    bqp = nc.dram_tensor("bqp", [P, 4], fp32, kind="ExternalInput")
    bvp = nc.dram_tensor("bvp", [P, 4], fp32, kind="ExternalInput")
    b1 = nc.dram_tensor("b1", [P, 4], fp32, kind="ExternalInput")
    b2 = nc.dram_tensor("b2", [1, OUT], fp32, kind="ExternalInput")
    out = nc.dram_tensor("out", [NBQ, OUT], fp32, kind="ExternalOutput")

    with tile.TileContext(nc) as tc:
        with (
            tc.tile_pool(name="const", bufs=1) as cw,
            tc.tile_pool(name="stream", bufs=2) as st,
            tc.tile_pool(name="small", bufs=2) as sm,
            tc.tile_pool(name="cast", bufs=8) as cp,
            tc.tile_pool(name="ps_small", bufs=2, space="PSUM") as ps_small,
            tc.tile_pool(name="ps_rowA", bufs=2, space="PSUM") as ps_rowA,
            tc.tile_pool(name="ps_rowB", bufs=2, space="PSUM") as ps_rowB,
        ):
            # ---- load weights / small inputs ----
            def mkload(pool, dt):
                def load(dram, shape):
                    t = pool.tile(shape, dt, tag=dram.name + "_sb")
                    nc.sync.dma_start(t[:], dram.ap())
                    return t
                return load

            # dense dummy bf16 matmul burst while the first DMAs are in
            # flight: trips the PE HAM clock-gate to 2.4 GHz before the
            # q-path matmul chain starts
            warm = cw.tile([P, 512], bf16, tag="warm")
            nc.gpsimd.memset(warm[:], 1.0)
            pwm = ps_small.tile([P, 512], fp32, tag="ps")
            for _ in range(14):
                nc.tensor.matmul(pwm[:], warm[:, 0:P], warm[:],
                                 start=True, stop=True, skip_group_check=True)

            loadw = mkload(cw, bf16)
            loadf = mkload(cw, fp32)
            sb_q0T = loadw(q0T, [P, 6, NBQ])
            sb_bqp = loadf(bqp, [P, 4])
            sb_bvp = loadf(bvp, [P, 4])
            sb_b1 = loadf(b1, [P, 4])
            sb_om = loadf(omask, [NBQ, OUT])

            # epilogue-only weights ride a different DMA queue (scalar) so
            # they never head-of-line-block the streaming loads on sync
            def loadw2(dram, shape):
                t = cw.tile(shape, bf16, tag=dram.name + "_sb")
                nc.scalar.dma_start(t[:], dram.ap())
                return t
            wvpT = loadw2(WvpT, [P, 6, H])
            wvT = loadw2(WvT, [P, 4, H])
            w1T = loadw2(W1T, [P, 8, H])
            w2T = loadw2(W2T, [P, 4, OUT])

            sb_b2b = cw.tile([NBQ, OUT], fp32, tag="b2b")
            nc.scalar.dma_start(sb_b2b[:], b2.ap().to_broadcast((NBQ, OUT)))
            sb_mb = cw.tile([NBQ, OUT], fp32, tag="mb")
            nc.vector.tensor_add(sb_mb[:], sb_om[:], sb_b2b[:])

            ones2 = cw.tile([P, 2], fp32, tag="ones2")
            nc.gpsimd.memset(ones2[:], 1.0)
            ones2b = cw.tile([P, 2], bf16, tag="ones2b")
            nc.vector.tensor_copy(ones2b[:], ones2[:])
            id10 = cw.tile([NBQ, NBQ], fp32, tag="id10")
            make_identity(nc, id10[:])
            # manual ring of product tiles; the ones-columns at 768:770 are
            # written once and never touched by the per-tile dot op, so
            # Z = sum(p) rides along in the second ebar matmul for free
            NPROD = 6
            prods = []
            for k in range(NPROD):
                pt = cw.tile([P, DW], bf16, tag=f"prodt{k}")
                nc.vector.tensor_copy(pt[:, D:D + 2], ones2b[:])
                prods.append(pt)

            # ---- prologue: q path -> cv_rows [10, 768] ----
            qT = cw.tile([P, 4, NBQ], bf16, tag="qT")
            cv_rows = cw.tile([NBQ, D], fp32, tag="cv_rows")
            with tc.tile_pool(name="prologue_w", bufs=2) as pw:
                def loadp(dram, shape):
                    t = pw.tile(shape, bf16, tag="wpro")
                    nc.sync.dma_start(t[:], dram.ap())
                    return t

                wqpT = loadp(WqpT, [P, 6, H])
                for mc in range(4):
                    pq = ps_small.tile([P, NBQ], fp32, tag="ps")
                    for kc in range(6):
                        nc.tensor.matmul(pq[:], wqpT[:, kc, ts(mc, P)],
                                         sb_q0T[:, kc, :],
                                         start=(kc == 0), stop=(kc == 5))
                    nc.vector.tensor_scalar(qT[:, mc, :], pq[:],
                                            sb_bqp[:, mc:mc + 1], None, Alu.add)

                wqT = loadp(WqT, [P, 4, H2])
                qqT = cw.tile([P, 8, NBQ], bf16, tag="qqT")
                for mc in range(8):
                    pq = ps_small.tile([P, NBQ], fp32, tag="ps")
                    for kc in range(4):
                        nc.tensor.matmul(pq[:], wqT[:, kc, ts(mc, P)],
                                         qT[:, kc, :],
                                         start=(kc == 0), stop=(kc == 3))
                    nc.scalar.copy(qqT[:, mc, :], pq[:])

                wk = loadp(Wk, [P, 8, H])
                ckT = cw.tile([P, 4, NBQ], bf16, tag="ckT")
                for mc in range(4):
                    pq = ps_small.tile([P, NBQ], fp32, tag="ps")
                    for kc in range(8):
                        nc.tensor.matmul(pq[:], wk[:, kc, ts(mc, P)],
                                         qqT[:, kc, :],
                                         start=(kc == 0), stop=(kc == 7))
                    nc.scalar.copy(ckT[:, mc, :], pq[:])

                wvp = loadp(Wvp, [P, 4, D])
                for nsl, nsz in ((slice(0, 512), 512), (slice(512, 768), 256)):
                    pq = ps_small.tile([NBQ, 512], fp32, tag="ps")
                    for kc in range(4):
                        nc.tensor.matmul(pq[:, :nsz], ckT[:, kc, :],
                                         wvp[:, kc, nsl],
                                         start=(kc == 0), stop=(kc == 3))
                    nc.scalar.copy(cv_rows[:, nsl], pq[:, :nsz])

            # rcv32 = 32/cv, used to undo the cv-scaling of the bf16 products
            rcv32 = cw.tile([NBQ, D], fp32, tag="rcv32")

            # stage each cv row to partition 0 and broadcast to 128
            # partitions up front (keeps the gpsimd queue clear during the
            # stream)
            cvb_all = cw.tile([P, NBQ, D], fp32, tag="cvb_all")
            for i in range(NBQ):
                cv_stage = sm.tile([1, D], fp32, tag="cv_stage")
                nc.scalar.dma_start(cv_stage[:], cv_rows[i:i + 1, :])
                nc.gpsimd.partition_broadcast(
                    cvb_all[:, i, :], cv_stage[0:1, :])

            # ---- streaming phase over the 10 (b,q) pairs ----
            # The per-bq normalize/evac chain runs one bq late, interleaved
            # into the next bq's stream so the DVE never stalls on the PE
            # finishing the psum row.
            ebar_rows = cw.tile([NBQ, D], fp32, tag="ebar_rows")

            def evac(pend):
                # psum -> (x * 1/Z) on ACT -> (* 32/cv) on GpSimd -> DMA out
                pi, prA, prB, rcv_stage = pend
                rz = sm.tile([1, 1], fp32, tag="rz")
                nc.vector.reciprocal(rz[:], prB[0:1, 256:257])
                tmp_row = sm.tile([1, D], fp32, tag="tmp_row")
                nc.scalar.activation(tmp_row[:, 0:512], prA[:], Act.Copy,
                                     scale=rz[0:1, 0:1])
                nc.scalar.activation(tmp_row[:, 512:768], prB[:, 0:256],
                                     Act.Copy, scale=rz[0:1, 0:1])
                row_sb = sm.tile([1, D], fp32, tag="row_sb")
                nc.gpsimd.tensor_tensor(row_sb[:], tmp_row[:],
                                        rcv_stage[0:1, :], Alu.mult)
                nc.gpsimd.dma_start(ebar_rows[pi:pi + 1, :], row_sb[:])

            pend = None
            for i in range(NBQ):
                eb = st.tile([P, SO, D], fp32, tag="eb")
                nc.sync.dma_start(eb[:, 0:SO // 2, :], ve.ap()[i, :, 0:SO // 2])
                nc.sync.dma_start(eb[:, SO // 2:, :], ve.ap()[i, :, SO // 2:])
                if i > 0:
                    rcv_stage = sm.tile([1, D], fp32, tag="rcv_stage")
                    nc.scalar.dma_start(rcv_stage[:], rcv32[i:i + 1, :])

                prA = ps_rowA.tile([1, 512], fp32, tag="prA")
                prB = ps_rowB.tile([1, 258], fp32, tag="prB")
                for t in range(SO):
                    prod = prods[(i * SO + t) % NPROD]
                    scc = sm.tile([P, 1], fp32, tag="scc")
                    nc.vector.scalar_tensor_tensor(
                        out=prod[:, 0:D],
                        in0=eb[:, t, :], scalar=INV_SQRT,
                        in1=cvb_all[:, i, :],
                        op0=Alu.mult, op1=Alu.mult,
                        accum_out=scc[:])
                    pcol = cp.tile([P, 1], bf16, tag="pcol")
                    nc.scalar.activation(pcol[:], scc[:], Act.Exp)
                    nc.tensor.matmul(prA[:], pcol[:], prod[:, 0:512],
                                     start=(t == 0), stop=(t == SO - 1))
                    nc.tensor.matmul(prB[:], pcol[:], prod[:, 512:D + 2],
                                     start=(t == 0), stop=(t == SO - 1))
                    if t == 2 and pend is not None:
                        evac(pend)
                if i == 0:
                    # deferred so the DVE starts streaming before the big
                    # reciprocal; only needed by evac(0) one bq later
                    nc.vector.reciprocal(rcv32[:], cv_rows[:])
                    nc.vector.tensor_scalar(rcv32[:], rcv32[:], 32.0,
                                            None, Alu.mult)
                    rcv_stage = sm.tile([1, D], fp32, tag="rcv_stage")
                    nc.scalar.dma_start(rcv_stage[:], rcv32[0:1, :])
                pend = (i, prA, prB, rcv_stage)
            evac(pend)

            # ---- epilogue on [*, 10] ----
            ebarT = cw.tile([P, 6, NBQ], bf16, tag="ebarT")
            for j in range(6):
                pt = ps_small.tile([P, NBQ], fp32, tag="ps")
                nc.tensor.transpose(pt[:], ebar_rows[:, ts(j, P)], id10[:])
                nc.scalar.copy(ebarT[:, j, :], pt[:])

            vbarT = cw.tile([P, 4, NBQ], bf16, tag="vbarT")
            for mc in range(4):
                pv = ps_small.tile([P, NBQ], fp32, tag="ps")
                for kc in range(6):
                    nc.tensor.matmul(pv[:], wvpT[:, kc, ts(mc, P)],
                                     ebarT[:, kc, :],
                                     start=(kc == 0), stop=(kc == 5))
                nc.vector.tensor_scalar(vbarT[:, mc, :], pv[:],
                                        sb_bvp[:, mc:mc + 1], None, Alu.add)

            pooledT = cw.tile([P, 4, NBQ], bf16, tag="pooledT")
            for mc in range(4):
                pv = ps_small.tile([P, NBQ], fp32, tag="ps")
                for kc in range(4):
                    nc.tensor.matmul(pv[:], wvT[:, kc, ts(mc, P)],
                                     vbarT[:, kc, :],
                                     start=(kc == 0), stop=(kc == 3))
                nc.scalar.copy(pooledT[:, mc, :], pv[:])

            xT = cw.tile([P, 4, NBQ], bf16, tag="xT")
            for mc in range(4):
                pv = ps_small.tile([P, NBQ], fp32, tag="ps")
                for kc in range(8):
                    rhs = pooledT[:, kc, :] if kc < 4 else qT[:, kc - 4, :]
                    nc.tensor.matmul(pv[:], w1T[:, kc, ts(mc, P)], rhs,
                                     start=(kc == 0), stop=(kc == 7))
                nc.scalar.activation(xT[:, mc, :], pv[:], Act.Relu,
                                     bias=sb_b1[:, mc:mc + 1])

            po = ps_small.tile([NBQ, OUT], fp32, tag="ps")
            for kc in range(4):
                nc.tensor.matmul(po[:], xT[:, kc, :], w2T[:, kc, :],
                                 start=(kc == 0), stop=(kc == 3))

            # + mask + b2, softmax over the 5 logits
            lg = sm.tile([NBQ, OUT], fp32, tag="lg")
            nc.vector.tensor_add(lg[:], po[:], sb_mb[:])
            ex = sm.tile([NBQ, OUT], fp32, tag="ex")
            nc.scalar.activation(ex[:], lg[:], Act.Exp)
            ssum = sm.tile([NBQ, 2], fp32, tag="ssum")
            nc.vector.tensor_reduce(ssum[:, 0:1], ex[:], Ax.X, Alu.add)
            nc.vector.reciprocal(ssum[:, 1:2], ssum[:, 0:1])
            res = sm.tile([NBQ, OUT], fp32, tag="res")
            nc.vector.tensor_scalar(res[:], ex[:], ssum[:, 1:2], None, Alu.mult)
            nc.sync.dma_start(out.ap(), res[:])

    nc.compile()
    return nc


def _get_compiled():
    global _COMPILED
    if _COMPILED is None:
        _COMPILED = _build()
    return _COMPILED


def _tile_lhst(w, bf):
    """[K, M] -> [128, K//128, M] partition-tiled, contiguous."""
    import ml_dtypes
    K, M = w.shape
    t = np.ascontiguousarray(w.reshape(K // P, P, M).transpose(1, 0, 2))
    return t.astype(ml_dtypes.bfloat16) if bf else t


def _tile_bias(b):
    return np.ascontiguousarray(b.reshape(-1, P).T)


def make_in_maps(video_enc, ques_enc, output_mask,
                 Wvp_, bvp_, Wqp_, bqp_, Wk_, Wv_, Wq_, W1_, b1_, W2_, b2_):
    import ml_dtypes
    ve_all = np.ascontiguousarray(video_enc, np.float32).reshape(
        80, SO, P, D).transpose(0, 2, 1, 3)
    q0 = np.ascontiguousarray(ques_enc[:, :, 0, :], np.float32).reshape(80, D)
    om = np.ascontiguousarray(output_mask, np.float32).reshape(80, OUT)

    common = dict(
        WqpT=_tile_lhst(Wqp_.T, True),    # [768, 512]
        WqT=_tile_lhst(Wq_.T, True),      # [512, 1024]
        Wk=_tile_lhst(Wk_, True),         # [1024, 512]
        Wvp=_tile_lhst(Wvp_, True),       # [512, 768]  (rhs layout)
        WvpT=_tile_lhst(Wvp_.T, True),    # [768, 512]
        WvT=_tile_lhst(Wv_.T, True),      # [512, 512]
        W1T=_tile_lhst(W1_.T, True),      # [1024, 512]
        W2T=_tile_lhst(W2_.T, True),      # [512, 5]
        bqp=np.float32(_tile_bias(bqp_)), bvp=np.float32(_tile_bias(bvp_)),
        b1=np.float32(_tile_bias(b1_)),
        b2=np.ascontiguousarray(b2_, np.float32).reshape(1, OUT),
    )

    in_maps = []
    for c in range(N_CORES):
        sl = slice(c * NBQ, (c + 1) * NBQ)
        m = dict(common)
        m["ve"] = np.ascontiguousarray(ve_all[sl])
        m["q0T"] = np.ascontiguousarray(
            q0[sl].T.reshape(6, P, NBQ).transpose(1, 0, 2)).astype(
                ml_dtypes.bfloat16)
        m["omask"] = om[sl]
        in_maps.append(m)
    return in_maps


def kernel(**inputs):
    global LAST_RESULTS
    from concourse.bass_utils import run_bass_kernel_spmd

    f = lambda k: np.asarray(inputs[k], np.float32)
    in_maps = make_in_maps(
        f("video_enc"), f("ques_enc"), f("output_mask"),
        f("Wvp"), f("bvp"), f("Wqp"), f("bqp"), f("Wk"), f("Wv"), f("Wq"),
        f("W1"), f("b1"), f("W2"), f("b2"))

    nc = _get_compiled()
    res = run_bass_kernel_spmd(nc, in_maps, core_ids=list(range(N_CORES)))
    LAST_RESULTS = res
    outs = np.concatenate([res.results[c]["out"] for c in range(N_CORES)], 0)
    return outs.reshape(16, 5, OUT).astype(np.float32)



# revision 7
# speedup vs baseline: 1.3473x; 1.3473x over previous
"""Trainium2 Bass kernel for nn_ClassifierAttn (single-query attention pooling).

Math restructuring (exact up to float assoc):
  Per (b,q) with e = video_enc[b,q] [S=2048, 768]:
    scores[s] = e[s] . cv,   cv = (G q0 + g0)/32,  G = Wvp^T Wk^T Wq Wqp
  (the bvp-dependent constant shift cancels in softmax), and
    pooled-path folds to  x1 = relu(M1 ebar + M2 q0 + c1),  logits = W2 x1 + b2
  with M1 = W1p Wv Wvp, M2 = W1q Wqp, c1 = W1p Wv bvp + W1q bqp + b1,
  ebar = softmax(scores)^T e.  All folds done on host in float64.

  The big tensor is touched by exactly two passes:
    (1) scores: DVE tensor_tensor mult (bf16, 2x mode) -> per-row reduce,
        split between DVE tensor_scalar+accum and ACT activation+accum
        so neither engine bottlenecks below the DMA stream rate.
    (2) pooling: PE matmuls p^T @ e accumulating one psum row per (b,q);
        host pads each row group to 772 with two ones-columns so
        Z = sum(p) rides along in the same psum row.

Wire format: video shipped bf16, host-tiled [NBQ, 128, 16, 772] (pad incl
ones); weights shipped bf16 after f64 folding. Stream DMA floor/core
~3.16MB/bq at ~358 GB/s = 8.8us; engines tuned to ~9us/bq.

Sharding: (B*QPV)=80 pairs, 10 per core across 8 cores; weights replicated.
"""

import numpy as np

P = 128
NBQ = 10          # (b,q) pairs per core
SO = 16           # S / P
S = 2048
D = 768
H = 512
OUT = 5
N_CORES = 8
DW = 772          # 768 data + 2 ones cols + 2 pad
NEB = 4           # eb ring depth
EXPG = 4          # exp batch size (tiles per ACT exp)
# tiles whose row-reduce runs on DVE tensor_scalar (rest on ACT):
TS_TILES = frozenset((0, 2, 5, 7, 9, 12, 14))

_COMPILED = None
LAST_RESULTS = None


def ts(i, size):
    return slice(i * size, (i + 1) * size)


def _build():
    import concourse.bass as bass  # noqa: F401
    import concourse.mybir as mybir
    import concourse.tile as tile
    from concourse import bacc
    from concourse.masks import make_identity

    fp32 = mybir.dt.float32
    bf16 = mybir.dt.bfloat16
    Alu = mybir.AluOpType
    Act = mybir.ActivationFunctionType

    nc = bacc.Bacc("TRN2", target_bir_lowering=False, debug=False,
                   num_devices=N_CORES)

    # ---- DRAM I/O (host-pre-tiled to DMA-contiguous layouts) ----
    ve = nc.dram_tensor("ve", [NBQ, P, SO, DW], bf16, kind="ExternalInput")
    q0T = nc.dram_tensor("q0T", [P, 6, NBQ], bf16, kind="ExternalInput")
    G32T = nc.dram_tensor("G32T", [P, 6, D], bf16, kind="ExternalInput")
    g0 = nc.dram_tensor("g0", [1, D], bf16, kind="ExternalInput")
    M1T = nc.dram_tensor("M1T", [P, 6, H], bf16, kind="ExternalInput")
    M2T = nc.dram_tensor("M2T", [P, 6, H], bf16, kind="ExternalInput")
    W2T = nc.dram_tensor("W2T", [P, 4, OUT], bf16, kind="ExternalInput")
    c1 = nc.dram_tensor("c1", [P, 4], fp32, kind="ExternalInput")
    mb = nc.dram_tensor("mb", [NBQ, OUT], fp32, kind="ExternalInput")
    out = nc.dram_tensor("out", [NBQ, OUT], fp32, kind="ExternalOutput")

    with tile.TileContext(nc) as tc:
        with (
            tc.tile_pool(name="const", bufs=1) as cw,
            tc.tile_pool(name="small", bufs=3) as sm,
            tc.tile_pool(name="scr", bufs=3) as scrp,
            tc.tile_pool(name="pcol", bufs=3) as cp,
            tc.tile_pool(name="ps_small", bufs=2, space="PSUM") as ps_small,
            tc.tile_pool(name="ps_rowA", bufs=2, space="PSUM") as ps_rowA,
            tc.tile_pool(name="ps_rowB", bufs=2, space="PSUM") as ps_rowB,
        ):
            # dense dummy bf16 matmul burst while the first DMAs are in
            # flight: trips the PE HAM clock-gate to 2.4 GHz before the
            # cv-path matmul chain starts
            warm = cw.tile([P, 512], bf16, tag="warm")
            nc.gpsimd.memset(warm[:], 1.0)
            pwm = ps_small.tile([P, 512], fp32, tag="ps")
            for _ in range(14):
                nc.tensor.matmul(pwm[:], warm[:, 0:P], warm[:],
                                 start=True, stop=True, skip_group_check=True)

            # early weights on the scalar HWDGE queue (cv path + classifier
            # biases); epilogue matrices ride the gpsimd SWDGE queue so they
            # never head-of-line-block anything
            def loadw(dram, shape, dt=bf16, eng=None):
                t = cw.tile(shape, dt, tag=dram.name + "_sb")
                (eng or nc.scalar).dma_start(t[:], dram.ap())
                return t

            sb_q0T = loadw(q0T, [P, 6, NBQ])
            sb_G32T = loadw(G32T, [P, 6, D])
            sb_g0 = loadw(g0, [1, D])
            sb_c1 = loadw(c1, [P, 4], fp32)
            sb_mb = loadw(mb, [NBQ, OUT], fp32)
            sb_M1T = loadw(M1T, [P, 6, H], eng=nc.gpsimd)
            sb_M2T = loadw(M2T, [P, 6, H], eng=nc.gpsimd)
            sb_W2T = loadw(W2T, [P, 4, OUT], eng=nc.gpsimd)

            ones10 = cw.tile([1, NBQ], bf16, tag="ones10")
            nc.gpsimd.memset(ones10[:], 1.0)
            id10 = cw.tile([NBQ, NBQ], fp32, tag="id10")
            make_identity(nc, id10[:])

            # eb ring (whole tile overwritten by each DMA, ones included)
            ebs = [cw.tile([P, SO, DW], bf16, name=f"eb{k}", tag=f"eb{k}")
                   for k in range(NEB)]

            # ---- prologue: cv_rows [10, 768] = q0 @ G32^T + g0 ----
            cvrows = cw.tile([NBQ, D], bf16, tag="cvrows")
            for nsl, nsz in ((slice(0, 512), 512), (slice(512, 768), 256)):
                pq = ps_small.tile([NBQ, 512], fp32, tag="ps")
                for kc in range(6):
                    nc.tensor.matmul(pq[:, 0:nsz], sb_q0T[:, kc, :],
                                     sb_G32T[:, kc, nsl],
                                     start=(kc == 0), stop=False)
                nc.tensor.matmul(pq[:, 0:nsz], ones10[:], sb_g0[:, nsl],
                                 start=False, stop=True)
                nc.scalar.copy(cvrows[:, nsl], pq[:, 0:nsz])

            # broadcast each cv row to all 128 partitions
            cvb_all = cw.tile([P, NBQ, D], bf16, tag="cvb_all")
            for i in range(NBQ):
                cv_stage = sm.tile([1, D], bf16, tag="cv_stage")
                nc.scalar.dma_start(cv_stage[:], cvrows[i:i + 1, :])
                nc.gpsimd.partition_broadcast(cvb_all[:, i, :],
                                              cv_stage[0:1, :])

            # ---- streaming phase ----
            # rotating [1, *] psum rows per (b,q) (matmul out must start at
            # partition 0); each bq's normalize/evac runs one bq late,
            # interleaved into the next bq's stream.
            ebar_rows = cw.tile([NBQ, D], fp32, tag="ebar_rows")

            # dead-store targets for the reduce side outputs
            scr2_ts = cw.tile([P, D], bf16, tag="scr2_ts")
            scr2_act = cw.tile([P, D], bf16, tag="scr2_act")

            def evac(pend):
                pi, prA, prB = pend
                rz = sm.tile([1, 1], fp32, tag="rz")
                nc.vector.reciprocal(rz[:], prB[0:1, 256:257])
                row = sm.tile([1, D], fp32, tag="row")
                nc.scalar.activation(row[:, 0:512], prA[:], Act.Copy,
                                     scale=rz[0:1, 0:1])
                nc.scalar.activation(row[:, 512:768], prB[:, 0:256],
                                     Act.Copy, scale=rz[0:1, 0:1])
                nc.gpsimd.dma_start(ebar_rows[pi:pi + 1, :], row[:])

            pend = None
            for i in range(NBQ):
                eb = ebs[i % NEB]
                if i == 0:
                    nc.sync.dma_start(eb[:, 0:SO // 2, :],
                                      ve.ap()[i, :, 0:SO // 2])
                    nc.sync.dma_start(eb[:, SO // 2:, :],
                                      ve.ap()[i, :, SO // 2:])
                else:
                    nc.sync.dma_start(eb[:], ve.ap()[i])

                prA = ps_rowA.tile([1, 512], fp32, tag="prA")
                prB = ps_rowB.tile([1, 258], fp32, tag="prB")
                for g in range(SO // EXPG):
                    scc = sm.tile([P, EXPG], fp32, tag="scc")
                    for j in range(EXPG):
                        t = g * EXPG + j
                        prod = scrp.tile([P, D], bf16, tag="prod")
                        nc.vector.tensor_tensor(prod[:], eb[:, t, 0:D],
                                                cvb_all[:, i, :], Alu.mult)
                        if t in TS_TILES:
                            nc.vector.tensor_scalar(
                                scr2_ts[:], prod[:], 1.0, 0.0, Alu.mult,
                                Alu.add, accum_out=scc[:, j:j + 1])
                        else:
                            nc.scalar.activation(
                                scr2_act[:], prod[:], Act.Copy,
                                accum_out=scc[:, j:j + 1])
                    pcol = cp.tile([P, EXPG], bf16, tag="pcol")
                    nc.scalar.activation(pcol[:], scc[:], Act.Exp)
                    for j in range(EXPG):
                        t = g * EXPG + j
                        nc.tensor.matmul(prA[:], pcol[:, j:j + 1],
                                         eb[:, t, 0:512],
                                         start=(t == 0), stop=(t == SO - 1))
                        nc.tensor.matmul(prB[:], pcol[:, j:j + 1],
                                         eb[:, t, 512:D + 2],
                                         start=(t == 0), stop=(t == SO - 1))
                    if g == 1 and pend is not None:
                        evac(pend)
                pend = (i, prA, prB)
            evac(pend)

            # ---- epilogue on [*, 10] ----
            ebarT = cw.tile([P, 6, NBQ], bf16, tag="ebarT")
            for j in range(6):
                pt = ps_small.tile([P, NBQ], fp32, tag="ps")
                nc.tensor.transpose(pt[:], ebar_rows[:, ts(j, P)], id10[:])
                nc.scalar.copy(ebarT[:, j, :], pt[:])

            x1T = cw.tile([P, 4, NBQ], bf16, tag="x1T")
            for mc in range(4):
                pv = ps_small.tile([P, NBQ], fp32, tag="ps")
                for kc in range(6):
                    nc.tensor.matmul(pv[:], sb_M1T[:, kc, ts(mc, P)],
                                     ebarT[:, kc, :],
                                     start=(kc == 0), stop=False)
                for kc in range(6):
                    nc.tensor.matmul(pv[:], sb_M2T[:, kc, ts(mc, P)],
                                     sb_q0T[:, kc, :],
                                     start=False, stop=(kc == 5))
                nc.scalar.activation(x1T[:, mc, :], pv[:], Act.Relu,
                                     bias=sb_c1[:, mc:mc + 1])

            po = ps_small.tile([NBQ, OUT], fp32, tag="ps")
            for kc in range(4):
                nc.tensor.matmul(po[:], x1T[:, kc, :], sb_W2T[:, kc, :],
                                 start=(kc == 0), stop=(kc == 3))

            # + (b2 + output_mask), softmax over the 5 logits
            lg = sm.tile([NBQ, OUT], fp32, tag="lg")
            nc.vector.tensor_add(lg[:], po[:], sb_mb[:])
            ex = sm.tile([NBQ, OUT], fp32, tag="ex")
            nc.scalar.activation(ex[:], lg[:], Act.Exp)
            ssum = sm.tile([NBQ, 2], fp32, tag="ssum")
            nc.vector.tensor_reduce(ssum[:, 0:1], ex[:],
                                    mybir.AxisListType.X, Alu.add)
            nc.vector.reciprocal(ssum[:, 1:2], ssum[:, 0:1])
            res = sm.tile([NBQ, OUT], fp32, tag="res")
            nc.vector.tensor_scalar(res[:], ex[:], ssum[:, 1:2], None,
                                    Alu.mult)
            nc.sync.dma_start(out.ap(), res[:])

    nc.compile()
    return nc


def _get_compiled():
    global _COMPILED
    if _COMPILED is None:
        _COMPILED = _build()
    return _COMPILED


def _tile_lhst(w):
    """[K, M] -> [128, K//128, M] partition-tiled, contiguous, bf16."""
    import ml_dtypes
    K, M = w.shape
    t = np.ascontiguousarray(
        np.asarray(w, np.float32).reshape(K // P, P, M).transpose(1, 0, 2))
    return t.astype(ml_dtypes.bfloat16)


def make_in_maps(video_enc, ques_enc, output_mask,
                 Wvp_, bvp_, Wqp_, bqp_, Wk_, Wv_, Wq_, W1_, b1_, W2_, b2_):
    import ml_dtypes

    # ---- weight folding in float64 (single rounding to bf16 at the end) ----
    Wvp = np.asarray(Wvp_, np.float64)
    Wqp = np.asarray(Wqp_, np.float64)
    Wk = np.asarray(Wk_, np.float64)
    Wv = np.asarray(Wv_, np.float64)
    Wq = np.asarray(Wq_, np.float64)
    W1 = np.asarray(W1_, np.float64)
    W2 = np.asarray(W2_, np.float64)
    bvp = np.asarray(bvp_, np.float64)
    bqp = np.asarray(bqp_, np.float64)
    b1 = np.asarray(b1_, np.float64)
    b2 = np.asarray(b2_, np.float64)

    core = Wvp.T @ (Wk.T @ (Wq @ Wqp))          # [768, 768]
    G32 = core / 32.0
    g0 = (Wvp.T @ (Wk.T @ (Wq @ bqp))) / 32.0   # [768]
    W1p, W1q = W1[:, :H], W1[:, H:]
    M1 = W1p @ (Wv @ Wvp)                        # [512, 768]
    M2 = W1q @ Wqp                               # [512, 768]
    c1v = W1p @ (Wv @ bvp) + W1q @ bqp + b1      # [512]

    # ---- big tensor: bf16, [80, 128, 16, 772] with ones cols baked in ----
    ve_b = np.asarray(video_enc, np.float32).reshape(
        80, SO, P, D).astype(ml_dtypes.bfloat16)
    arr = np.empty((80, P, SO, DW), ml_dtypes.bfloat16)
    arr[..., :D] = ve_b.transpose(0, 2, 1, 3)
    arr[..., D:D + 2] = ml_dtypes.bfloat16(1.0)
    arr[..., D + 2:] = 0

    q0 = np.ascontiguousarray(
        np.asarray(ques_enc, np.float32)[:, :, 0, :]).reshape(80, D)
    om = np.asarray(output_mask, np.float64).reshape(80, OUT)
    mb_all = np.float32(om + b2)

    common = dict(
        G32T=_tile_lhst(G32.T),                  # [768, 768] d-major
        g0=np.asarray(g0, np.float32).reshape(1, D).astype(ml_dtypes.bfloat16),
        M1T=_tile_lhst(M1.T),                    # [768, 512]
        M2T=_tile_lhst(M2.T),                    # [768, 512]
        W2T=_tile_lhst(W2.T),                    # [512, 5]
        c1=np.float32(np.ascontiguousarray(c1v.reshape(-1, P).T)),
    )

    in_maps = []
    for c in range(N_CORES):
        sl = slice(c * NBQ, (c + 1) * NBQ)
        m = dict(common)
        m["ve"] = arr[sl]
        m["q0T"] = np.ascontiguousarray(
            q0[sl].T.reshape(6, P, NBQ).transpose(1, 0, 2)).astype(
                ml_dtypes.bfloat16)
        m["mb"] = np.ascontiguousarray(mb_all[sl])
        in_maps.append(m)
    return in_maps


def kernel(**inputs):
    global LAST_RESULTS
    from concourse.bass_utils import run_bass_kernel_spmd

    in_maps = make_in_maps(
        inputs["video_enc"], inputs["ques_enc"], inputs["output_mask"],
        inputs["Wvp"], inputs["bvp"], inputs["Wqp"], inputs["bqp"],
        inputs["Wk"], inputs["Wv"], inputs["Wq"],
        inputs["W1"], inputs["b1"], inputs["W2"], inputs["b2"])

    nc = _get_compiled()
    res = run_bass_kernel_spmd(nc, in_maps, core_ids=list(range(N_CORES)))
    LAST_RESULTS = res
    outs = np.concatenate([res.results[c]["out"] for c in range(N_CORES)], 0)
    return outs.reshape(16, 5, OUT).astype(np.float32)
